# revision 1
# baseline (speedup 1.0000x reference)
"""Trainium2 Bass kernel for nn_DARM_43267500540057 (dense_cnn).

Sharding: pure data-parallel — 8 images (batch B=8), one per NeuronCore.
No collectives.

Per-core layout: feature maps live in SBUF as [C=128 partitions, 130*130]
float32r with the 128x128 image at grid [1..128]^2 (1-px zero halo).
A 3x3 "same" conv is 9 tap-shifted matmuls accumulating in PSUM: the tap
shift (dy,dx) is a free-dim offset dy*130+dx on the input AP. float32r
(TF32-class) runs the PE at 1 cycle/row vs 4 for float32.

Stages (per core):
  head     : im2row27 (host-built bf16, streamed) @ W27         -> f [128ch]
  h1a,h1b  : 9-tap conv + bias + relu                           -> f
  SE1      : channel means via ACT accum_out on h1b evictions; the
             sigmoid scale folds into w_h2a's input-channel rows
  h2a,h2b  : conv + bias + relu
  SE2      : same, folded into w_tail
  tail     : conv + b_tail -> wgt [72ch]
  dyn      : dyn3[(t,c)] = sum_k kernel[k,t] * wgt[k]  (matmul K=72,M=75)
  combine  : tmp = dyn3 * X75 (host-built bf16 im2row of reflect-padded
             x, streamed), out[c] = sum_t tmp[(t,c)] (matmul K=75,M=3)
             with a row-strided rhs AP -> final [3,128*128] layout.

Eviction garbage at halo-ring columns is re-zeroed per chunk (DVE) so
consumers never stall on a whole-buffer barrier. Streams use large DMA
segments and constants are host-packed to minimize DMA instruction count.
"""

import ml_dtypes
import numpy as np

import concourse.bacc as bacc
import concourse.bass as bass
import concourse.mybir as mybir
import concourse.tile as tile
from concourse.bass_utils import run_bass_kernel_spmd

F32 = mybir.dt.float32
F32R = mybir.dt.float32r
BF16 = mybir.dt.bfloat16
AF = mybir.ActivationFunctionType

# geometry
H = W = 128
G = 130                    # trunk grid stride (128 + 2*halo)
NG = G * G                 # 16900
Q0, Q1 = G + 1, 128 * G + 128 + 1   # matmul output q-range [131, 16769)
NQ = Q1 - Q0               # 16638
CHUNK = 512
CHUNKS = [(Q0 + i * CHUNK, min(CHUNK, Q1 - (Q0 + i * CHUNK)))
          for i in range((NQ + CHUNK - 1) // CHUNK)]   # 32x512 + 1x254
NCH = len(CHUNKS)
SEG = 4096                 # stream-DMA segment (elements)
SEGS = [(Q0 + i * SEG, min(SEG, Q1 - (Q0 + i * SEG)))
        for i in range((NQ + SEG - 1) // SEG)]         # 4x4096 + 1x254
# head: row-aligned chunks (2 grid rows each) so evictions can skip the
# halo-ring columns with strided APs
HCH = [(131, 259)] + [(130 + 260 * r, 260) for r in range(1, 63)] + [(16510, 259)]
HSEGS = [(HCH[16 * s][0], HCH[16 * s + 15][0] + HCH[16 * s + 15][1])
         for s in range(4)]
K = 72
NPIX = H * W

# packed-constant column layout (f32 pack, [128, CP_COLS])
CP_B1A, CP_B1B, CP_B2A, CP_B2B, CP_BTAIL = 0, 1, 2, 3, 4
CP_SE1B1, CP_SE1B2, CP_SE2B1, CP_SE2B2 = 5, 6, 7, 8
CP_SE1W1 = 9            # [128, 8]
CP_SE2W1 = 17           # [128, 8]
CP_SE1W2 = 25           # [8, 128]
CP_SE2W2 = 153          # [8, 128]
CP_COLS = 281


def _build_bass():
    nc = bacc.Bacc("TRN2", target_bir_lowering=False)

    # ---- DRAM I/O (per-core shapes; host preprocesses layouts) ----
    d_im27 = nc.dram_tensor("im27", [27, NG], BF16, kind="ExternalInput")
    d_x75 = nc.dram_tensor("x75", [75, NQ], BF16, kind="ExternalInput")
    d_w27 = nc.dram_tensor("w27", [27, 128], BF16, kind="ExternalInput")
    d_bhead = nc.dram_tensor("bhead", [128, 1], F32, kind="ExternalInput")
    d_wt = {}
    for name in ("w1a", "w1b", "w2a", "w2b"):
        d_wt[name] = nc.dram_tensor(name, [128, 9 * 128], F32R, kind="ExternalInput")
    d_wtail = nc.dram_tensor("wtail", [128, 9 * K], F32R, kind="ExternalInput")
    d_cpack = nc.dram_tensor("cpack", [128, CP_COLS], F32, kind="ExternalInput")
    d_kpack = nc.dram_tensor("kpack", [75, 75], F32R, kind="ExternalInput")
    d_s99 = nc.dram_tensor("s99", [75, 396], BF16, kind="ExternalInput")
    d_out = nc.dram_tensor("out", [3, NPIX], F32, kind="ExternalOutput")

    with tile.TileContext(nc) as tc:
        with (
            tc.tile_pool(name="fmap", bufs=2) as fmap_pool,
            tc.tile_pool(name="wts", bufs=1) as wpool,
            tc.tile_pool(name="seg", bufs=2) as segpool,
            tc.tile_pool(name="stream", bufs=4) as spool,
            tc.tile_pool(name="small", bufs=1) as small,
            tc.tile_pool(name="cpsum", bufs=2, space="PSUM") as cpsum,
            tc.tile_pool(name="hdpsum", bufs=3, space="PSUM") as hdpsum,
            tc.tile_pool(name="opsum", bufs=2, space="PSUM") as opsum,
            tc.tile_pool(name="sepsum", bufs=1, space="PSUM") as sepsum,
        ):
            # ---- head-critical weights only (rest loads after head issue) ----
            w27 = wpool.tile_from(d_w27[:, :], name="w27")
            bhead = wpool.tile_from(d_bhead[:, :], name="bhead")
            cpack = wpool.tile_from(d_cpack[:, :], name="cpack")
            col = lambda i: cpack[:, i:i + 1]

            def ring_chunk(dst, qs, n):
                # zero eviction garbage at halo-ring cols within [qs, qs+n)
                v = dst.rearrange("p (h w) -> p h w", w=G)
                r0 = -(-(qs - 129) // G); r1 = -(-(qs + n - 129) // G)
                if r1 > r0:
                    nc.vector.memset(v[:, r0:r1, 129:130].bitcast(F32), 0.0)
                r0 = -(-qs // G); r1 = -(-(qs + n) // G)
                if r1 > r0:
                    nc.vector.memset(v[:, r0:r1, 0:1].bitcast(F32), 0.0)

            def conv9_chunk(ci, src, dst, w_lhsT, bias_ap, relu, acc=None,
                            cout=128, ring=True):
                    qs, n = CHUNKS[ci]
                    ps = cpsum.tile([128, CHUNK], F32, tag="cps")
                    for t in range(9):
                        dt = (t // 3 - 1) * G + (t % 3 - 1)
                        nc.tensor.matmul(
                            out=ps[:cout, :n],
                            lhsT=w_lhsT[:, t * cout:(t + 1) * cout],
                            rhs=src[:, qs + dt: qs + dt + n],
                            start=(t == 0), stop=(t == 8),
                        )
                    kw = {}
                    if acc is not None:
                        kw["accum_out"] = acc[:, ci:ci + 1]
                    nc.scalar.activation(
                        out=dst[:cout, qs:qs + n], in_=ps[:cout, :n],
                        func=AF.Relu if relu else AF.Identity,
                        bias=bias_ap, **kw)
                    if ring:
                        ring_chunk(dst, qs, n)

            def conv9(src, dst, w_lhsT, bias_ap, relu, acc=None, cout=128,
                      ring=True):
                """dst[:cout, q] = act(sum_taps w_t.T @ src[:, q+dt] + b)"""
                for ci in range(NCH):
                    conv9_chunk(ci, src, dst, w_lhsT, bias_ap, relu, acc,
                                cout, ring)

            def se_fold(acc, w1t, b1, w2t, b2, w_in, w_out, wcols):
                ssum = small.tile([128, 1], F32, name=f"ssum{wcols}")
                nc.vector.reduce_sum(out=ssum[:, 0:1], in_=acc[:, 0:NCH],
                                     axis=mybir.AxisListType.X)
                ps1 = sepsum.tile([8, 1], F32, tag="seps")
                nc.tensor.matmul(out=ps1[0:8, 0:1], lhsT=w1t,
                                 rhs=ssum[:, 0:1], start=True, stop=True)
                s1 = small.tile([8, 1], F32, name=f"s1_{wcols}")
                nc.scalar.activation(out=s1[0:8, 0:1], in_=ps1[0:8, 0:1],
                                     func=AF.Relu, bias=b1)
                ps2 = sepsum.tile([128, 1], F32, tag="seps")
                nc.tensor.matmul(out=ps2[:, 0:1], lhsT=w2t,
                                 rhs=s1[0:8, 0:1], start=True, stop=True)
                s2 = small.tile([128, 1], F32, name=f"s2_{wcols}")
                nc.scalar.activation(out=s2[:, 0:1], in_=ps2[:, 0:1],
                                     func=AF.Sigmoid, bias=b2)
                cw = wcols // 9
                for t in range(9):
                    nc.vector.tensor_scalar_mul(
                        out=w_out[:, t * cw:(t + 1) * cw],
                        in0=w_in[:, t * cw:(t + 1) * cw],
                        scalar1=s2[:, 0:1])

            # ---- head: K=27 matmul over seg-streamed im2row ----
            # Row-aligned chunks; evictions write only image cols (strided),
            # so halo stays zero from the one-time memsets below.
            f = fmap_pool.tile([128, NG], F32R, tag="fmap", name="f_head")
            nc.vector.memset(f[:, 0:131].bitcast(F32), 0.0)
            nc.vector.memset(f[:, 16769:NG].bitcast(F32), 0.0)
            rg = f[:, 129:129 + 129 * G].rearrange("p (h w) -> p h w", w=G)
            nc.vector.memset(rg[:, :, 0:2].bitcast(F32), 0.0)
            fv = f.rearrange("p (h w) -> p h w", w=G)
            f1 = fmap_pool.tile([128, NG], F32R, tag="fmap", name="f_h1a")
            nc.vector.memset(f1[:, 0:131].bitcast(F32), 0.0)
            nc.vector.memset(f1[:, 16769:NG].bitcast(F32), 0.0)
            wsb = {}
            for si, (ss, se) in enumerate(HSEGS):
                sn = se - ss
                imseg = segpool.tile([27, 4160], BF16, tag="seg")
                nc.sync.dma_start(out=imseg[:, :sn], in_=d_im27[:, ss:ss + se - ss])
                if si == 0:
                    wsb["w1a"] = wpool.tile_from(d_wt["w1a"][:, :], name="sb_w1a")
                for ci in range(16 * si, 16 * si + 16):
                    qs, n = HCH[ci]
                    off = qs - ss
                    ps = hdpsum.tile([128, 260], F32, tag="hd")
                    nc.tensor.matmul(out=ps[:, :n], lhsT=w27[:, :],
                                     rhs=imseg[0:27, off:off + n],
                                     start=True, stop=True)
                    pv = ps.rearrange("p (a b) -> p a b", b=G)
                    pcol = slice(0, 128) if ci == 0 else slice(1, 129)
                    row = 2 * ci + 1
                    if ci % 2 == 0:
                        nc.scalar.activation(out=fv[:, row:row + 2, 1:129],
                                             in_=pv[:, :, pcol],
                                             func=AF.Identity, bias=bhead)
                    else:
                        nc.vector.tensor_scalar_add(out=fv[:, row:row + 2, 1:129],
                                                    in0=pv[:, :, pcol],
                                                    scalar1=bhead)

            # ---- remaining resident weights/constants ----
            for n in ("w1b", "w2a", "w2b"):
                wsb[n] = wpool.tile_from(d_wt[n][:, :], name=f"sb_{n}")
            wtail = wpool.tile_from(d_wtail[:, :], name="wtail")
            kpack = wpool.tile_from(d_kpack[:, :], name="kpack")
            w2a_s = wpool.tile([128, 9 * 128], F32R)   # SE1-scaled w2a
            wtail_s = wpool.tile([128, 9 * K], F32R)   # SE2-scaled wtail
            s99sb = wpool.tile_from(d_s99[:, :], name="s99sb")
            k3 = kpack[0:K, 0:75]
            s99 = [s99sb[0:75, 99 * i: 99 * (i + 1)] for i in range(4)]

            # ---- trunk ----
            conv9(f, f1, wsb["w1a"], col(CP_B1A), relu=True)

            se1_acc = small.tile([128, NCH], F32)
            f2 = fmap_pool.tile([128, NG], F32R, tag="fmap", name="f_h1b")
            conv9(f1, f2, wsb["w1b"], col(CP_B1B), relu=True, acc=se1_acc)
            se_fold(se1_acc, cpack[:, CP_SE1W1:CP_SE1W1 + 8], cpack[0:8, CP_SE1B1:CP_SE1B1 + 1],
                    cpack[0:8, CP_SE1W2:CP_SE1W2 + 128], col(CP_SE1B2),
                    wsb["w2a"], w2a_s, 9 * 128)

            f3 = fmap_pool.tile([128, NG], F32R, tag="fmap", name="f_h2a")
            conv9(f2, f3, w2a_s, col(CP_B2A), relu=True)

            se2_acc = small.tile([128, NCH], F32)
            f4 = fmap_pool.tile([128, NG], F32R, tag="fmap", name="f_h2b")
            conv9(f3, f4, wsb["w2b"], col(CP_B2B), relu=True, acc=se2_acc)
            se_fold(se2_acc, cpack[:, CP_SE2W1:CP_SE2W1 + 8], cpack[0:8, CP_SE2B1:CP_SE2B1 + 1],
                    cpack[0:8, CP_SE2W2:CP_SE2W2 + 128], col(CP_SE2B2),
                    wtail, wtail_s, 9 * K)

            wgt = fmap_pool.tile([128, NG], F32R, tag="fmap", name="wgt")
            conv9(f4, wgt, wtail_s, cpack[0:K, CP_BTAIL:CP_BTAIL + 1],
                  relu=False, cout=K, ring=False)

            # ---- dyn + elementwise (seg-streamed X75) ----
            # muls split: 2/3 DVE (PSUM-direct), 1/3 ACT-evict + GPSIMD
            tmp = fmap_pool.tile([128, NG], BF16, tag="fmap", name="tmp75")
            ci = 0
            for ss, sn in SEGS:
                xseg = segpool.tile([75, SEG], BF16, tag="seg")
                nc.sync.dma_start(out=xseg[:, :sn],
                                  in_=d_x75[:, ss - Q0: ss - Q0 + sn])
                while ci < NCH and CHUNKS[ci][0] < ss + sn:
                    qs, n = CHUNKS[ci]
                    off = qs - ss
                    psd = hdpsum.tile([75, CHUNK], F32, tag="hd")
                    nc.tensor.matmul(out=psd[0:75, :n], lhsT=k3,
                                     rhs=wgt[0:K, qs:qs + n],
                                     start=True, stop=True)
                    if ci % 3 == 2:
                        dync = spool.tile([75, CHUNK], BF16, tag="dync")
                        nc.scalar.activation(out=dync[0:75, :n],
                                             in_=psd[0:75, :n], func=AF.Copy)
                        nc.gpsimd.tensor_mul(out=tmp[0:75, qs:qs + n],
                                             in0=dync[0:75, :n],
                                             in1=xseg[0:75, off:off + n])
                    else:
                        nc.vector.tensor_mul(out=tmp[0:75, qs:qs + n],
                                             in0=psd[0:75, :n],
                                             in1=xseg[0:75, off:off + n])
                    ci += 1

            # ---- combine: 4 row-strided MMs accumulate into one bank at
            # partition offsets 0/32/64/96 (selector lhsT), 1 evict per 4 ----
            out3 = fmap_pool.tile([99, 8 * CHUNK], F32, tag="fmap",
                                  name="out3")
            tmpv = tmp.rearrange("p (h w) -> p h w", w=G)
            for j in range(8):
                pso = opsum.tile([99, CHUNK], F32, tag="ops")
                for i in range(4):
                    r = 4 * j + i
                    nc.tensor.matmul(
                        out=pso[0:99, :], lhsT=s99[i],
                        rhs=tmpv[0:75, 1 + 4 * r: 5 + 4 * r, 1:129],
                        start=(i == 0), stop=(i == 3))
                nc.scalar.activation(out=out3[0:99, 512 * j: 512 * (j + 1)],
                                     in_=pso[0:99, :], func=AF.Copy)
            dv = d_out.rearrange("c (g i e) -> c g i e", i=4, e=CHUNK)
            ov = out3.rearrange("p (g e) -> p g e", e=CHUNK)
            for i in range(4):
                nc.sync.dma_start(out=dv[:, :, i, :],
                                  in_=ov[32 * i: 32 * i + 3, :, :])

    nc.compile()
    return nc


_NC_CACHE = None


def _get_nc():
    global _NC_CACHE
    if _NC_CACHE is None:
        _NC_CACHE = _build_bass()
    return _NC_CACHE


def _prep_core_inputs(b, x, w_head, b_head, w_h1a, b_h1a, w_h1b, b_h1b,
                      du1_w1, du1_b1, du1_w2, du1_b2,
                      w_h2a, b_h2a, w_h2b, b_h2b,
                      du2_w1, du2_b1, du2_w2, du2_b2,
                      w_tail, b_tail, kernel):
    from numpy.lib.stride_tricks import sliding_window_view

    f32 = np.float32
    bf16 = ml_dtypes.bfloat16
    xb = np.asarray(x[b], dtype=f32)

    # im2row of zero-padded x over the 130-grid: row (t*3+c), t=ty*3+tx
    xz2 = np.pad(xb, ((0, 0), (2, 2), (2, 2)))
    sw2 = sliding_window_view(xz2, (130, 130), axis=(1, 2))  # [3,3,3,130,130]
    im27 = np.ascontiguousarray(
        sw2.transpose(1, 2, 0, 3, 4).reshape(27, NG)).astype(bf16)

    # im2row of reflect-padded x at 5x5 taps, columns indexed by q-Q0
    xp = np.pad(xb, ((0, 0), (2, 2), (2, 2)), mode="reflect")
    sw = sliding_window_view(xp, (5, 5), axis=(1, 2))  # [3,128,128,5,5]
    a75 = sw.transpose(3, 4, 0, 1, 2).reshape(75, 128, 128)  # [(dy dx c),h,w]
    xbig = np.zeros((75, 128 * G), dtype=f32)
    xbig.reshape(75, 128, G)[:, :, 0:128] = a75
    x75 = np.ascontiguousarray(xbig[:, :NQ]).astype(bf16)

    def lhsT(w):  # [O,I,3,3] -> [I, 9*O] tap-major
        o = w.shape[0]
        return np.ascontiguousarray(
            w.transpose(1, 2, 3, 0).reshape(w.shape[1], 9 * o), dtype=f32)

    cpack = np.zeros((128, CP_COLS), dtype=f32)
    cpack[:, CP_B1A] = b_h1a
    cpack[:, CP_B1B] = b_h1b
    cpack[:, CP_B2A] = b_h2a
    cpack[:, CP_B2B] = b_h2b
    cpack[0:K, CP_BTAIL] = b_tail
    cpack[0:8, CP_SE1B1] = du1_b1
    cpack[:, CP_SE1B2] = du1_b2
    cpack[0:8, CP_SE2B1] = du2_b1
    cpack[:, CP_SE2B2] = du2_b2
    cpack[:, CP_SE1W1:CP_SE1W1 + 8] = du1_w1.reshape(8, 128).T / 16384.0
    cpack[:, CP_SE2W1:CP_SE2W1 + 8] = du2_w1.reshape(8, 128).T / 16384.0
    cpack[0:8, CP_SE1W2:CP_SE1W2 + 128] = du1_w2.reshape(128, 8).T
    cpack[0:8, CP_SE2W2:CP_SE2W2 + 128] = du2_w2.reshape(128, 8).T

    kpack = np.zeros((75, 75), dtype=f32)
    kpack[0:K, 0:75] = np.repeat(kernel.reshape(K, 25), 3, axis=1)
    s99 = np.zeros((75, 396), dtype=f32)
    s75 = np.tile(np.eye(3, dtype=f32), (25, 1))
    for i in range(4):
        s99[0:75, 99 * i + 32 * i: 99 * i + 32 * i + 3] = s75

    return {
        "im27": im27, "x75": x75,
        "w27": np.ascontiguousarray(
            w_head.transpose(2, 3, 1, 0).reshape(27, 128)).astype(bf16),
        "bhead": np.ascontiguousarray(np.asarray(b_head, f32).reshape(128, 1)),
        "w1a": lhsT(w_h1a), "w1b": lhsT(w_h1b),
        "w2a": lhsT(w_h2a), "w2b": lhsT(w_h2b), "wtail": lhsT(w_tail),
        "cpack": cpack, "kpack": kpack,
        "s99": s99.astype(ml_dtypes.bfloat16),
    }


def kernel(**inputs):
    n_cores = 8
    nc = _get_nc()
    in_maps = [_prep_core_inputs(b, **inputs) for b in range(n_cores)]
    res = run_bass_kernel_spmd(nc, in_maps, core_ids=list(range(n_cores)))
    out = np.stack([r["out"].reshape(3, H, W) for r in res.results], axis=0)
    return out.astype(np.float32)


if __name__ == "__main__":
    import reference
    inputs = {k: np.asarray(v) for k, v in reference.setup_inputs().items()}
    got = kernel(**inputs)
    exp = np.asarray(reference.reference(**inputs))
    err = np.abs(got - exp).max() / (np.abs(exp).max() + 1e-30)
    print("Relative error:", err)



# revision 5
# speedup vs baseline: 1.1089x; 1.1089x over previous
"""Trainium2 Bass kernel for nn_DARM_43267500540057 (dense_cnn).

Sharding: pure data-parallel — 8 images (batch B=8), one per NeuronCore.
No collectives.

Per-core layout: feature maps live in SBUF as [C=128 partitions, 130*130]
bf16 with the 128x128 image at grid [1..128]^2 (1-px zero halo).

The five 3x3 conv layers (h1a/h1b/h2a/h2b/tail) use 1-D Winograd F(2,3)
along y: per chunk of `nr` tile-rows (2 output rows each),
  V_p = Bt-combos of 4 y-shifted f rows (DVE, bf16, 4 ops)
  M_p = sum_dx W'_{p,dx}^T V_p(x+dx)   (12 matmuls, 3-dx PSUM accum,
        rhs bf16 -> 1 cyc/row; lhsT f32r holds G-transformed weights)
  y0 = relu(M0+M1+M2+b), y1 = relu(M1-M2-M3+b)  (Pool pair-adds, DVE
        final adds, ACT bias+relu+SE-accum eviction, strided to image
        cols so halos stay zero)
This is 12 matmul-cols/output-row-pair vs 18 for direct 3x3 — PE time
for the trunk drops ~1.5x; transforms ride on otherwise-idle engines.

Other stages (as baseline): head via streamed im2row27 @ W27; SE scales
fold into the next layer's input-channel rows; dyn = kernel-basis matmul
K=72->75 times streamed reflect-pad im2row (x75); combine via selector
matmuls into [99]-partition out3, DMA'd out in two halves.

All PSUM = one pool, 4 tags x 2 bufs of [128,512]f32 (8 banks), shared
by head/trunk/dyn/combine phases.
"""

import ml_dtypes
import numpy as np

import concourse.bacc as bacc
import concourse.bass as bass
import concourse.mybir as mybir
import concourse.tile as tile
from concourse.bass_utils import run_bass_kernel_spmd

F32 = mybir.dt.float32
F32R = mybir.dt.float32r
BF16 = mybir.dt.bfloat16
AF = mybir.ActivationFunctionType

# geometry
H = W = 128
G = 130                    # grid stride (128 + 2*halo)
NG = G * G                 # 16900
Q0, Q1 = G + 1, 128 * G + 128 + 1   # dyn q-range [131, 16769)
NQ = Q1 - Q0               # 16638
CHUNK = 512
CHUNKS = [(Q0 + i * CHUNK, min(CHUNK, Q1 - (Q0 + i * CHUNK)))
          for i in range((NQ + CHUNK - 1) // CHUNK)]   # 32x512 + 1x254
NCH = len(CHUNKS)
SEG = 4096                 # stream-DMA segment (elements)
SEGS = [(Q0 + i * SEG, min(SEG, Q1 - (Q0 + i * SEG)))
        for i in range((NQ + SEG - 1) // SEG)]         # 4x4096 + 1x254
# head: row-aligned chunks (2 grid rows each)
HCH = [(131, 259)] + [(130 + 260 * r, 260) for r in range(1, 63)] + [(16510, 259)]
HSEGS = [(HCH[8 * s][0], HCH[8 * s + 7][0] + HCH[8 * s + 7][1])
         for s in range(8)]
# winograd tile-row chunks (r0, nr): 20x3 + 2x2 rows = 64 tile rows
RCH = [(3 * i, 3) for i in range(20)] + [(60, 2), (62, 2)]
NRCH = len(RCH)
K = 72
NPIX = H * W

# packed-constant column layout (f32 pack, [128, CP_COLS])
CP_B1A, CP_B1B, CP_B2A, CP_B2B, CP_BTAIL = 0, 1, 2, 3, 4
CP_SE1B1, CP_SE1B2, CP_SE2B1, CP_SE2B2 = 5, 6, 7, 8
CP_SE1W1 = 9            # [128, 8]
CP_SE2W1 = 17           # [128, 8]
CP_SE1W2 = 25           # [8, 128]
CP_SE2W2 = 153          # [8, 128]
CP_COLS = 281


def _build_bass():
    nc = bacc.Bacc("TRN2", target_bir_lowering=False)

    # ---- DRAM I/O (per-core shapes; host preprocesses layouts) ----
    d_im27 = nc.dram_tensor("im27", [27, NG], BF16, kind="ExternalInput")
    d_x75 = nc.dram_tensor("x75", [75, NQ], BF16, kind="ExternalInput")
    d_w27 = nc.dram_tensor("w27", [27, 128], BF16, kind="ExternalInput")
    d_bhead = nc.dram_tensor("bhead", [128, 1], F32, kind="ExternalInput")
    d_wt = {}
    for name in ("w1a", "w1b", "w2a", "w2b"):
        d_wt[name] = nc.dram_tensor(name, [128, 12 * 128], F32R, kind="ExternalInput")
    d_wtail = nc.dram_tensor("wtail", [128, 12 * K], F32R, kind="ExternalInput")
    d_cpack = nc.dram_tensor("cpack", [128, CP_COLS], F32, kind="ExternalInput")
    d_kpack = nc.dram_tensor("kpack", [75, 75], F32R, kind="ExternalInput")
    d_s99 = nc.dram_tensor("s99", [75, 396], BF16, kind="ExternalInput")
    d_out = nc.dram_tensor("out", [3, NPIX], F32, kind="ExternalOutput")

    with tile.TileContext(nc) as tc:
        with (
            tc.tile_pool(name="fmap", bufs=2) as fmap_pool,
            tc.tile_pool(name="wts", bufs=1) as wpool,
            tc.tile_pool(name="seg", bufs=2) as segpool,
            tc.tile_pool(name="vts", bufs=2) as vpool,
            tc.tile_pool(name="stream", bufs=4) as spool,
            tc.tile_pool(name="small", bufs=1) as small,
            tc.tile_pool(name="ps", bufs=2, space="PSUM") as psum,
        ):
            # ---- head-critical weights only (rest loads after head issue) ----
            w27 = wpool.tile_from(d_w27[:, :], name="w27")
            bhead = wpool.tile_from(d_bhead[:, :], name="bhead")
            cpack = wpool.tile_from(d_cpack[:, :], name="cpack")
            col = lambda i: cpack[:, i:i + 1]

            def ring_memset(t):
                # zero full halo ring: top row+, bottom row+, per-row edges
                nc.vector.memset(t[:, 0:131], 0.0)
                nc.vector.memset(t[:, 16769:NG], 0.0)
                rg = t[:, 129:129 + 129 * G].rearrange("p (h w) -> p h w", w=G)
                nc.vector.memset(rg[:, :, 0:2], 0.0)

            def conv_wino(src, dst, w_lhsT, bias_ap, relu, acc=None, cout=128):
                """1-D y-Winograd F(2,3) 3x3 conv: dst = act(conv(src)+b).

                src/dst: [128, NG] bf16 grid tiles. w_lhsT: [128, 12*cout]
                f32r, columns (p*3+dx)*cout+o of G-transformed weights.
                """
                sv = src.rearrange("p (h2 two w) -> p h2 two w", two=2, w=G)
                dv = dst.rearrange("p (h2 two w) -> p h2 two w", two=2, w=G)
                for ci, (r0, nr) in enumerate(RCH):
                    n = nr * G
                    n2 = nr * 128
                    # V transform (bf16): rows f(2r-1+i) = grid(2r+i)
                    vt = vpool.tile([128, 4, 3 * G + 2], BF16, tag="vt")
                    nc.vector.memset(vt[:, :, 0:1], 0.0)
                    nc.vector.memset(vt[:, :, 1 + n:2 + n], 0.0)
                    e = lambda r, c: sv[:, r:r + nr, c, :]   # rows step 2
                    nc.vector.tensor_sub(out=vt[:, 0, 1:1 + n], in0=e(r0, 0),
                                         in1=e(r0 + 1, 0))
                    nc.vector.tensor_add(out=vt[:, 1, 1:1 + n], in0=e(r0, 1),
                                         in1=e(r0 + 1, 0))
                    nc.vector.tensor_sub(out=vt[:, 2, 1:1 + n],
                                         in0=e(r0 + 1, 0), in1=e(r0, 1))
                    nc.vector.tensor_sub(out=vt[:, 3, 1:1 + n], in0=e(r0, 1),
                                         in1=e(r0 + 1, 1))
                    # 12 matmuls: M_p = sum_dx W'_{p,dx}^T V_p(.+dx)
                    ps = []
                    for p in range(4):
                        mp = psum.tile([128, 512], F32, tag=f"m{p}")
                        for dx in range(3):
                            nc.tensor.matmul(
                                out=mp[:cout, :n],
                                lhsT=w_lhsT[:, (p * 3 + dx) * cout:
                                            (p * 3 + dx + 1) * cout],
                                rhs=vt[:, p, dx:dx + n],
                                start=(dx == 0), stop=(dx == 2))
                        ps.append(mp)
                    # image-col views of M (skip per-row halo cols)
                    mv = [mp[:cout, 0:n].rearrange("p (r w) -> p r w", w=G)
                          [:, :, 1:129] for mp in ps]
                    t0 = spool.tile([128, 384], F32, tag="t0")
                    t1 = spool.tile([128, 384], F32, tag="t1")
                    s0 = spool.tile([128, 384], F32, tag="s0")
                    s1 = spool.tile([128, 384], F32, tag="s1")
                    nc.gpsimd.tensor_add(out=t0[:cout, :n2], in0=mv[1], in1=mv[2])
                    nc.gpsimd.tensor_sub(out=t1[:cout, :n2], in0=mv[1], in1=mv[2])
                    nc.vector.tensor_add(out=s0[:cout, :n2], in0=mv[0],
                                         in1=t0[:cout, :n2])
                    nc.vector.tensor_sub(out=s1[:cout, :n2], in0=t1[:cout, :n2],
                                         in1=mv[3])
                    func = AF.Relu if relu else AF.Identity
                    kw0 = {"accum_out": acc[:, 2 * ci:2 * ci + 1]} if acc is not None else {}
                    kw1 = {"accum_out": acc[:, 2 * ci + 1:2 * ci + 2]} if acc is not None else {}
                    nc.scalar.activation(
                        out=dv[:cout, r0:r0 + nr, 1, 1:129],
                        in_=s0[:cout, :n2], func=func, bias=bias_ap, **kw0)
                    nc.scalar.activation(
                        out=dv[:cout, r0 + 1:r0 + 1 + nr, 0, 1:129],
                        in_=s1[:cout, :n2], func=func, bias=bias_ap, **kw1)

            def se_fold(acc, w1t, b1, w2t, b2, w_in, w_out):
                ssum = small.tile([128, 1], F32, name=f"ssum{id(acc)}")
                nc.vector.reduce_sum(out=ssum[:, 0:1], in_=acc[:, 0:2 * NRCH],
                                     axis=mybir.AxisListType.X)
                ps1 = psum.tile([128, 512], F32, tag="m0")
                nc.tensor.matmul(out=ps1[0:8, 0:1], lhsT=w1t,
                                 rhs=ssum[:, 0:1], start=True, stop=True)
                s1 = small.tile([8, 1], F32, name=f"s1_{id(acc)}")
                nc.scalar.activation(out=s1[0:8, 0:1], in_=ps1[0:8, 0:1],
                                     func=AF.Relu, bias=b1)
                ps2 = psum.tile([128, 512], F32, tag="m1")
                nc.tensor.matmul(out=ps2[:, 0:1], lhsT=w2t,
                                 rhs=s1[0:8, 0:1], start=True, stop=True)
                s2 = small.tile([128, 1], F32, name=f"s2_{id(acc)}")
                nc.scalar.activation(out=s2[:, 0:1], in_=ps2[:, 0:1],
                                     func=AF.Sigmoid, bias=b2)
                nc.vector.tensor_scalar_mul(out=w_out[:, :], in0=w_in[:, :],
                                            scalar1=s2[:, 0:1])

            # ---- head: K=27 matmul over seg-streamed im2row ----
            f = fmap_pool.tile([128, NG], BF16, tag="fmap", name="f_head")
            ring_memset(f)
            fv = f.rearrange("p (h w) -> p h w", w=G)
            f1 = fmap_pool.tile([128, NG], BF16, tag="fmap", name="f_h1a")
            ring_memset(f1)
            wsb = {}
            for si, (ss, se) in enumerate(HSEGS):
                sn = se - ss
                imseg = segpool.tile([27, 2080], BF16, tag="seg")
                nc.sync.dma_start(out=imseg[:, :sn], in_=d_im27[:, ss:se])
                if si == 0:
                    wsb["w1a"] = wpool.tile_from(d_wt["w1a"][:, :], name="sb_w1a")
                for ci in range(8 * si, 8 * si + 8):
                    qs, n = HCH[ci]
                    off = qs - ss
                    ps = psum.tile([128, 512], F32, tag=f"m{ci % 4}")
                    nc.tensor.matmul(out=ps[:, :n], lhsT=w27[:, :],
                                     rhs=imseg[0:27, off:off + n],
                                     start=True, stop=True)
                    pv = ps[:, 0:2 * G].rearrange("p (a b) -> p a b", b=G)
                    pcol = slice(0, 128) if ci == 0 else slice(1, 129)
                    row = 2 * ci + 1
                    if ci % 2 == 0:
                        nc.scalar.activation(out=fv[:, row:row + 2, 1:129],
                                             in_=pv[:, 0:2, pcol],
                                             func=AF.Identity, bias=bhead)
                    else:
                        nc.vector.tensor_scalar_add(out=fv[:, row:row + 2, 1:129],
                                                    in0=pv[:, 0:2, pcol],
                                                    scalar1=bhead)

            # ---- remaining resident weights/constants ----
            for nme in ("w1b", "w2a", "w2b"):
                wsb[nme] = wpool.tile_from(d_wt[nme][:, :], name=f"sb_{nme}")
            wtail = wpool.tile_from(d_wtail[:, :], name="wtail")
            kpack = wpool.tile_from(d_kpack[:, :], name="kpack")
            w2a_s = wpool.tile([128, 12 * 128], F32R)   # SE1-scaled w2a
            wtail_s = wpool.tile([128, 12 * K], F32R)   # SE2-scaled wtail
            s99sb = wpool.tile_from(d_s99[:, :], name="s99sb")
            k3 = kpack[0:K, 0:75]
            s99 = [s99sb[0:75, 99 * i: 99 * (i + 1)] for i in range(4)]

            # ---- trunk ----
            conv_wino(f, f1, wsb["w1a"], col(CP_B1A), relu=True)

            se1_acc = small.tile([128, 2 * NRCH], F32)
            f2 = fmap_pool.tile([128, NG], BF16, tag="fmap", name="f_h1b")
            conv_wino(f1, f2, wsb["w1b"], col(CP_B1B), relu=True, acc=se1_acc)
            se_fold(se1_acc, cpack[:, CP_SE1W1:CP_SE1W1 + 8],
                    cpack[0:8, CP_SE1B1:CP_SE1B1 + 1],
                    cpack[0:8, CP_SE1W2:CP_SE1W2 + 128], col(CP_SE1B2),
                    wsb["w2a"], w2a_s)

            f3 = fmap_pool.tile([128, NG], BF16, tag="fmap", name="f_h2a")
            conv_wino(f2, f3, w2a_s, col(CP_B2A), relu=True)

            se2_acc = small.tile([128, 2 * NRCH], F32)
            f4 = fmap_pool.tile([128, NG], BF16, tag="fmap", name="f_h2b")
            conv_wino(f3, f4, wsb["w2b"], col(CP_B2B), relu=True, acc=se2_acc)
            se_fold(se2_acc, cpack[:, CP_SE2W1:CP_SE2W1 + 8],
                    cpack[0:8, CP_SE2B1:CP_SE2B1 + 1],
                    cpack[0:8, CP_SE2W2:CP_SE2W2 + 128], col(CP_SE2B2),
                    wtail, wtail_s)

            wgt = fmap_pool.tile([128, NG], BF16, tag="fmap", name="wgt")
            conv_wino(f4, wgt, wtail_s, cpack[0:K, CP_BTAIL:CP_BTAIL + 1],
                      relu=False, cout=K)

            # ---- dyn + elementwise (seg-streamed X75) ----
            # muls split: 2/3 DVE (PSUM-direct), 1/3 ACT-evict + GPSIMD
            tmp = fmap_pool.tile([128, NG], BF16, tag="fmap", name="tmp75")
            ci = 0
            for ss, sn in SEGS:
                xseg = segpool.tile([75, SEG], BF16, tag="xseg")
                nc.sync.dma_start(out=xseg[:, :sn],
                                  in_=d_x75[:, ss - Q0: ss - Q0 + sn])
                while ci < NCH and CHUNKS[ci][0] < ss + sn:
                    qs, n = CHUNKS[ci]
                    off = qs - ss
                    psd = psum.tile([128, 512], F32, tag=f"m{ci % 4}")
                    nc.tensor.matmul(out=psd[0:75, :n], lhsT=k3,
                                     rhs=wgt[0:K, qs:qs + n],
                                     start=True, stop=True)
                    if ci % 3 == 2:
                        dync = spool.tile([75, CHUNK], BF16, tag="dync")
                        nc.scalar.activation(out=dync[0:75, :n],
                                             in_=psd[0:75, :n], func=AF.Copy)
                        nc.gpsimd.tensor_mul(out=tmp[0:75, qs:qs + n],
                                             in0=dync[0:75, :n],
                                             in1=xseg[0:75, off:off + n])
                    else:
                        nc.vector.tensor_mul(out=tmp[0:75, qs:qs + n],
                                             in0=psd[0:75, :n],
                                             in1=xseg[0:75, off:off + n])
                    ci += 1

            # ---- combine: 4 row-strided MMs accumulate into one bank at
            # partition offsets 0/32/64/96 (selector lhsT), 1 evict per 4;
            # output DMA'd in two 4-group halves ----
            out3 = fmap_pool.tile([99, 8 * CHUNK], F32, tag="fmap",
                                  name="out3")
            tmpv = tmp.rearrange("p (h w) -> p h w", w=G)
            dv4 = d_out.rearrange("c (g i e) -> c g i e", i=4, e=CHUNK)
            ov = out3.rearrange("p (g e) -> p g e", e=CHUNK)
            for j in range(8):
                pso = psum.tile([128, 512], F32, tag=f"m{j % 4}")
                for i in range(4):
                    r = 4 * j + i
                    nc.tensor.matmul(
                        out=pso[0:99, :], lhsT=s99[i],
                        rhs=tmpv[0:75, 1 + 4 * r: 5 + 4 * r, 1:129],
                        start=(i == 0), stop=(i == 3))
                nc.scalar.activation(out=out3[0:99, 512 * j: 512 * (j + 1)],
                                     in_=pso[0:99, :], func=AF.Copy)
                if j == 3:
                    for i in range(4):
                        nc.sync.dma_start(out=dv4[:, 0:4, i, :],
                                          in_=ov[32 * i: 32 * i + 3, 0:4, :])
            for i in range(4):
                nc.sync.dma_start(out=dv4[:, 4:8, i, :],
                                  in_=ov[32 * i: 32 * i + 3, 4:8, :])

    nc.compile()
    return nc


_NC_CACHE = None


def _get_nc():
    global _NC_CACHE
    if _NC_CACHE is None:
        _NC_CACHE = _build_bass()
    return _NC_CACHE


def _prep_core_inputs(b, x, w_head, b_head, w_h1a, b_h1a, w_h1b, b_h1b,
                      du1_w1, du1_b1, du1_w2, du1_b2,
                      w_h2a, b_h2a, w_h2b, b_h2b,
                      du2_w1, du2_b1, du2_w2, du2_b2,
                      w_tail, b_tail, kernel):
    from numpy.lib.stride_tricks import sliding_window_view

    f32 = np.float32
    bf16 = ml_dtypes.bfloat16
    xb = np.asarray(x[b], dtype=f32)

    # im2row of zero-padded x over the 130-grid: row (t*3+c), t=ty*3+tx
    xz2 = np.pad(xb, ((0, 0), (2, 2), (2, 2)))
    sw2 = sliding_window_view(xz2, (130, 130), axis=(1, 2))  # [3,3,3,130,130]
    im27 = np.ascontiguousarray(
        sw2.transpose(1, 2, 0, 3, 4).reshape(27, NG)).astype(bf16)

    # im2row of reflect-padded x at 5x5 taps, columns indexed by q-Q0
    xp = np.pad(xb, ((0, 0), (2, 2), (2, 2)), mode="reflect")
    sw = sliding_window_view(xp, (5, 5), axis=(1, 2))  # [3,128,128,5,5]
    a75 = sw.transpose(3, 4, 0, 1, 2).reshape(75, 128, 128)  # [(dy dx c),h,w]
    xbig = np.zeros((75, 128 * G), dtype=f32)
    xbig.reshape(75, 128, G)[:, :, 0:128] = a75
    x75 = np.ascontiguousarray(xbig[:, :NQ]).astype(bf16)

    # y-Winograd G-transform: lhsT[c, (p*3+dx)*O + o] = sum_ty Gy[p,ty] w[o,c,ty,dx]
    Gy = np.array([[1, 0, 0], [.5, .5, .5], [.5, -.5, .5], [0, 0, 1]], dtype=f32)

    def wlhsT(w):  # [O,I,3,3] -> [I, 12*O], (p, dx, o) column order
        o = w.shape[0]
        arr = np.einsum("pt,octx->cpxo", Gy, np.asarray(w, f32))
        return np.ascontiguousarray(arr.reshape(w.shape[1], 12 * o))

    cpack = np.zeros((128, CP_COLS), dtype=f32)
    cpack[:, CP_B1A] = b_h1a
    cpack[:, CP_B1B] = b_h1b
    cpack[:, CP_B2A] = b_h2a
    cpack[:, CP_B2B] = b_h2b
    cpack[0:K, CP_BTAIL] = b_tail
    cpack[0:8, CP_SE1B1] = du1_b1
    cpack[:, CP_SE1B2] = du1_b2
    cpack[0:8, CP_SE2B1] = du2_b1
    cpack[:, CP_SE2B2] = du2_b2
    cpack[:, CP_SE1W1:CP_SE1W1 + 8] = du1_w1.reshape(8, 128).T / 16384.0
    cpack[:, CP_SE2W1:CP_SE2W1 + 8] = du2_w1.reshape(8, 128).T / 16384.0
    cpack[0:8, CP_SE1W2:CP_SE1W2 + 128] = du1_w2.reshape(128, 8).T
    cpack[0:8, CP_SE2W2:CP_SE2W2 + 128] = du2_w2.reshape(128, 8).T

    kpack = np.zeros((75, 75), dtype=f32)
    kpack[0:K, 0:75] = np.repeat(kernel.reshape(K, 25), 3, axis=1)
    s99 = np.zeros((75, 396), dtype=f32)
    s75 = np.tile(np.eye(3, dtype=f32), (25, 1))
    for i in range(4):
        s99[0:75, 99 * i + 32 * i: 99 * i + 32 * i + 3] = s75

    return {
        "im27": im27, "x75": x75,
        "w27": np.ascontiguousarray(
            w_head.transpose(2, 3, 1, 0).reshape(27, 128)).astype(bf16),
        "bhead": np.ascontiguousarray(np.asarray(b_head, f32).reshape(128, 1)),
        "w1a": wlhsT(w_h1a), "w1b": wlhsT(w_h1b),
        "w2a": wlhsT(w_h2a), "w2b": wlhsT(w_h2b), "wtail": wlhsT(w_tail),
        "cpack": cpack, "kpack": kpack,
        "s99": s99.astype(ml_dtypes.bfloat16),
    }


def kernel(**inputs):
    n_cores = 8
    nc = _get_nc()
    in_maps = [_prep_core_inputs(b, **inputs) for b in range(n_cores)]
    res = run_bass_kernel_spmd(nc, in_maps, core_ids=list(range(n_cores)))
    out = np.stack([r["out"].reshape(3, H, W) for r in res.results], axis=0)
    return out.astype(np.float32)


if __name__ == "__main__":
    import reference
    inputs = {k: np.asarray(v) for k, v in reference.setup_inputs().items()}
    got = kernel(**inputs)
    exp = np.asarray(reference.reference(**inputs))
    err = np.abs(got - exp).max() / (np.abs(exp).max() + 1e-30)
    print("Relative error:", err)
